# revision 16
# baseline (speedup 1.0000x reference)
"""Trainium2 Bass kernel for the DCL contrastive loss (nn_DCL_11776800325979).

Reference computation:
    feats = concat([z1, z2])                       # [8192, 128]
    cos = (feats @ feats.T) / max(|f_i||f_j|, eps) # [8192, 8192]
    cos[i,i] = -9e15 ; s = cos / 0.1
    pos_i = s[i, (i+4096) % 8192]
    neg = s with the pos column also masked to -9e15
    loss = mean(-pos_i + logsumexp(neg, axis=-1))

Strategy (8 NeuronCores, data-parallel over rows):
  Each core receives feats rolled by -c*1024 rows and computes the loss rows
  for *local* rows 0..1023 against all 8192 columns.  Rolling both the row
  and column index space by the same amount preserves the self-mask (i==j)
  and the positive-pair offset ((i+4096) mod 8192), so a single NEFF runs
  SPMD on all 8 cores with no dynamic addressing.

  On-chip per core:
    warmup (per 2048-col chunk g):
      - DMA-transpose bf16 chunk -> Graw^T [128, 2048]
      - row-major contiguous bf16 load -> Fr [128, 16, 128] (16 rows/part)
      - DVE: SQ = Fr*Fr, reduce_sum over d -> SS [128,16]; reciprocal -> ISS
      - ACT: rn = sqrt(ISS * S^2)  (bf16, = S/|f|)
      - gpsimd: flatten rn -> row [1,2048]; partition_broadcast -> BC;
        partition-shift Graw[64:128] -> Ghi (for the fp8 split-K layout)
      - DVE: G2[:,a,:] = Graw_half_a * BC  -> fp8e4 (values = S * f/|f|)
    cos phase per (g, row-block b):
      - 4x matmul fp8 DoubleRow [64,2,128]x[64,2,512] -> PSUM [128, 2048]
        (= S^2 * cos block)
      - diag masking (g==0) / pos extract+mask (g==2) on DVE
      - exp+rowsum: ACT Exp(scale=10/S^2, accum_out) for most blocks;
        a few g in {1,3} blocks use a DVE Schraudolph exp (affine->int32
        bitcast) + reduce_sum to offload the scalar engine.
    tail: RS = sum over g; LSE = Ln(RS); loss rows = -10/S^2*POS + LSE.
  Host: sum the 8x[128,8] row losses, divide by 8192.

HBM traffic ~4MB in + 4KB out per core; the 8192^2 similarity matrix never
leaves PSUM.  Steady state is balanced between the scalar engine (Exp) and
DVE (Schraudolph exp + masking), with the PE at ~50% on fp8 matmuls.
"""

import numpy as np

D = 128          # feature dim (= contraction dim)
N2 = 8192        # 2N rows
NCORES = 8
RPC = N2 // NCORES          # rows per core = 1024
RB = RPC // 128             # row blocks per core = 8
CG = 4                      # column chunks of 2048
CGW = N2 // CG              # chunk width = 2048
RPP = CGW // 128            # rows per partition in the row-major load = 16
NEG = -9.0e15               # matches torch masked_fill value in reference
S_FP8 = 64.0                # fp8 scale: G stores S * normalized feats
INV_TEMP = 10.0
SCHRAUD_A = 12102203.1616   # 2^23 / ln 2
SCHRAUD_B = 1065353216.0 - 482208.0  # 127*2^23 - C (C calibrated, zero bias)

USE_FP8 = False
USE_RECIP = False   # DVE reciprocal + ACT sqrt; else Newton rsqrt on DVE
USE_TTR = False     # fused tensor_tensor_reduce for the POS extract
USE_SCHRAUD = False  # offload some exp blocks to DVE (Schraudolph)
MAGIC = 0x5F3759DF  # fast inverse sqrt seed (Newton path)
# (g, b) blocks whose exp+rowsum runs on DVE via Schraudolph instead of ACT.
# Only g in {1,3} blocks (no masked diagonal) are eligible.
DVE_BLOCKS = {(3, b) for b in range(RB)} | {(1, b) for b in (1, 3, 5)}

_CACHE = {}
LAST_RESULTS = None


def _build():
    """Build + compile the SPMD Bass kernel once; cache the Bass object."""
    if "nc" in _CACHE:
        return _CACHE["nc"]

    from contextlib import ExitStack

    import concourse.bass as bass  # noqa: F401  (AP helpers)
    import concourse.mybir as mybir
    import concourse.tile as tile
    from concourse import bacc

    f32 = mybir.dt.float32
    i32 = mybir.dt.int32
    bf16 = mybir.dt.bfloat16
    fp8 = mybir.dt.float8e4
    AF = mybir.ActivationFunctionType
    ALU = mybir.AluOpType
    X = mybir.AxisListType.X
    PM = mybir.MatmulPerfMode

    nc = bacc.Bacc(
        "TRN2",
        target_bir_lowering=False,
        debug=False,
        enable_asserts=False,
        num_devices=NCORES,
    )

    featsb = nc.dram_tensor("featsb", [N2, D], bf16, kind="ExternalInput").ap()
    eye_d = nc.dram_tensor("eye", [128, 128], f32, kind="ExternalInput").ap()
    out_d = nc.dram_tensor("loss_rows", [128, RB], f32, kind="ExternalOutput").ap()

    gdt = fp8 if USE_FP8 else bf16
    act_scale = INV_TEMP / (S_FP8 * S_FP8) if USE_FP8 else INV_TEMP
    sqrt_scale = S_FP8 * S_FP8 if USE_FP8 else 1.0
    schraud_a = SCHRAUD_A * act_scale

    with tile.TileContext(nc) as tc, ExitStack() as ctx:
        consts = ctx.enter_context(tc.tile_pool(name="consts", bufs=1))
        gpool = ctx.enter_context(tc.tile_pool(name="G", bufs=1))
        fpool = ctx.enter_context(tc.tile_pool(name="F", bufs=2))
        sqpool = ctx.enter_context(tc.tile_pool(name="SQ", bufs=2))
        bcpool = ctx.enter_context(tc.tile_pool(name="BC", bufs=2))
        stat = ctx.enter_context(tc.tile_pool(name="stat", bufs=1))
        epool = ctx.enter_context(tc.tile_pool(name="E", bufs=3))
        tpool = ctx.enter_context(tc.tile_pool(name="T", bufs=2))
        ppool = ctx.enter_context(tc.tile_pool(name="P", bufs=2, space="PSUM"))

        def _dep(after, before, reason):
            a = getattr(after, "ins", after)
            b = getattr(before, "ins", before)
            tile.add_dep_helper(a, b, reason=reason)

        eye = consts.tile([128, 128], f32)
        nc.gpsimd.dma_start(eye[:], eye_d[:, :])
        ones_row = consts.tile([1, 128], bf16)
        nc.vector.memset(ones_row[:], 1.0)
        if not USE_RECIP:
            magicT = consts.tile([128, RPP], i32)
            nc.vector.memset(magicT[:], MAGIC)
            c15 = consts.tile([128, RPP], f32)
            nc.vector.memset(c15[:], 1.5)

        rdt = bf16
        SS = stat.tile([128, RPP * CG], f32)    # per-row |f|^2
        ISS = stat.tile([128, RPP * CG], f32)   # 1/|f|^2
        RN = stat.tile([128, RPP * CG], rdt)    # S/|f|
        ROW1 = stat.tile([1, N2], rdt)          # rn flattened on partition 0
        SUMS = stat.tile([128, RB * CG], f32)   # row sumexp per (b, g)
        POS = stat.tile([128, RB], f32)         # S^2 * cos of the positive pair

        Graw = [
            gpool.tile([128, CGW], bf16, tag=f"Gr{g}", name=f"Gr{g}")
            for g in range(CG)
        ]
        if USE_FP8:
            Ghi = [
                gpool.tile([64, CGW], bf16, tag=f"Gh{g}", name=f"Gh{g}")
                for g in range(CG)
            ]
            G2 = [
                gpool.tile([64, 2, CGW], fp8, tag=f"G{g}", name=f"G{g}")
                for g in range(CG)
            ]
        else:
            G2 = [
                gpool.tile([128, CGW], bf16, tag=f"G{g}", name=f"G{g}")
                for g in range(CG)
            ]

        # ---- warmup: loads + norms + normalized G ----
        # stage 1: all DMAs first (two HBM streams in parallel)
        Frs = []
        for g in range(CG):
            c0 = g * CGW
            teng = nc.sync if g % 2 == 0 else nc.scalar
            teng.dma_start(
                Graw[g][:], featsb[c0:c0 + CGW, :], transpose=True
            )
            Fr = fpool.tile([128, CGW], bf16, tag=f"F{g}", name=f"Fr{g}")
            nc.gpsimd.dma_start(
                Fr[:].rearrange("p (r d) -> p r d", d=D),
                featsb[c0:c0 + CGW, :].rearrange("(p r) d -> p r d", p=128),
            )
            Frs.append(Fr)
        # stage 2: row sums-of-squares + Newton rsqrt per chunk (DVE)
        for g in range(CG):
            lo, hi = g * RPP, (g + 1) * RPP
            SQ = sqpool.tile([128, CGW], bf16, tag="SQ", name="SQ")
            nc.vector.tensor_mul(SQ[:], Frs[g][:], Frs[g][:])
            nc.vector.reduce_sum(
                SS[:, lo:hi],
                SQ[:].rearrange("p (r d) -> p r d", d=D), axis=X,
            )
            x = SS[:, lo:hi]
            y = sqpool.tile([128, RPP], f32, tag="nw_y", name="nw_y")
            t = sqpool.tile([128, RPP], f32, tag="nw_t", name="nw_t")
            nc.vector.tensor_scalar(
                y[:].bitcast(i32), x.bitcast(i32), 1, None,
                op0=ALU.logical_shift_right,
            )
            nc.vector.tensor_sub(y[:].bitcast(i32), magicT[:],
                                 y[:].bitcast(i32))
            for _ in range(3):
                nc.vector.tensor_mul(t[:], y[:], y[:])
                nc.vector.tensor_mul(t[:], t[:], x)
                nc.vector.scalar_tensor_tensor(
                    t[:], t[:], -0.5, c15[:], ALU.mult, ALU.add
                )
                nc.vector.tensor_mul(y[:], y[:], t[:])
            nc.vector.tensor_copy(RN[:, lo:hi], y[:])
        # stage 3: flatten rn -> row, PE outer-product broadcast -> PSUM,
        # scale Graw -> normalized bf16 G
        for g in range(CG):
            c0 = g * CGW
            lo, hi = g * RPP, (g + 1) * RPP
            nc.gpsimd.dma_start(
                ROW1[0:1, c0:c0 + CGW].rearrange("q (p r) -> q p r", p=128),
                RN[:, lo:hi],
            )
            BCp = ppool.tile([128, CGW], f32, tag="P", name="BCp")
            for t in range(4):
                nc.tensor.matmul(
                    BCp[:, t * 512:(t + 1) * 512],
                    ones_row[:],
                    ROW1[0:1, c0 + t * 512:c0 + (t + 1) * 512],
                    start=True, stop=True,
                )
            nc.vector.tensor_mul(G2[g][:], Graw[g][:], BCp[:])

        # ---- cos blocks + exp ----
        act_insts = []
        for g in range(CG):
            for b in range(RB):
                P = ppool.tile([128, CGW], f32, tag="P", name="P")
                for t in range(4):
                    if USE_FP8:
                        nc.tensor.matmul(
                            P[:, t * 512:(t + 1) * 512],
                            G2[0][:, :, b * 128:(b + 1) * 128],
                            G2[g][:, :, t * 512:(t + 1) * 512],
                            start=True, stop=True,
                            perf_mode=PM.DoubleRow,
                        )
                    else:
                        nc.tensor.matmul(
                            P[:, t * 512:(t + 1) * 512],
                            G2[0][:, b * 128:(b + 1) * 128],
                            G2[g][:, t * 512:(t + 1) * 512],
                            start=True, stop=True,
                        )
                off = b * 128
                if g == 2:
                    # positive pair: diagonal of cols 4096+b*128
                    dscr = sqpool.tile([128, 128], f32, tag="dscr",
                                       name="dscr")
                    if USE_TTR:
                        nc.vector.tensor_tensor_reduce(
                            dscr[:], P[:, off:off + 128], eye[:], 1.0, 0.0,
                            ALU.mult, ALU.add, accum_out=POS[:, b:b + 1],
                        )
                    else:
                        nc.vector.tensor_mul(dscr[:], P[:, off:off + 128],
                                             eye[:])
                        nc.vector.reduce_sum(POS[:, b:b + 1], dscr[:], axis=X)
                if g == 0 or g == 2:
                    nc.vector.scalar_tensor_tensor(
                        P[:, off:off + 128], eye[:], NEG, P[:, off:off + 128],
                        ALU.mult, ALU.add,
                    )
                k = b * CG + g
                if USE_SCHRAUD and (g, b) in DVE_BLOCKS:
                    # Schraudolph: exp(act_scale*x) ~ bitcast(A'*x + B)
                    T = tpool.tile([128, CGW], i32, tag="T", name="T")
                    nc.vector.tensor_scalar(
                        T[:], P[:], schraud_a, SCHRAUD_B,
                        op0=ALU.mult, op1=ALU.add,
                    )
                    red_i = nc.vector.reduce_sum(
                        SUMS[:, k:k + 1], T[:].bitcast(f32), axis=X
                    )
                    act_insts.append(red_i)
                else:
                    E = epool.tile([128, CGW], bf16, tag="E", name="E")
                    act_i = nc.scalar.activation(
                        E[:], P[:], AF.Exp, scale=act_scale,
                        accum_out=SUMS[:, k:k + 1],
                    )
                    act_insts.append(act_i)

        # ---- tail: lse + loss rows ----
        RS = stat.tile([128, RB], f32)
        red_i = nc.vector.reduce_sum(
            RS[:], SUMS[:].rearrange("p (b g) -> p b g", g=CG), axis=X
        )
        for a in act_insts:
            _dep(red_i, a, "RS reads per-(b,g) sums")
        LSE = stat.tile([128, RB], f32)
        nc.scalar.activation(LSE[:], RS[:], AF.Ln)
        LOSS = stat.tile([128, RB], f32)
        nc.vector.scalar_tensor_tensor(
            LOSS[:], POS[:], -act_scale, LSE[:], ALU.mult, ALU.add
        )
        nc.gpsimd.dma_start(out_d[:, :], LOSS[:])

    nc.compile()
    _CACHE["nc"] = nc
    return nc


def kernel(z1: np.ndarray, z2: np.ndarray) -> np.ndarray:
    global LAST_RESULTS
    import ml_dtypes
    from concourse.bass_utils import run_bass_kernel_spmd

    z1 = np.ascontiguousarray(np.asarray(z1, dtype=np.float32))
    z2 = np.ascontiguousarray(np.asarray(z2, dtype=np.float32))
    feats = np.concatenate([z1, z2], axis=0)
    feats_bf = feats.astype(ml_dtypes.bfloat16)
    eye = np.eye(128, dtype=np.float32)

    in_maps = []
    for c in range(NCORES):
        fb = np.ascontiguousarray(np.roll(feats_bf, -c * RPC, axis=0))
        in_maps.append({"featsb": fb, "eye": eye})

    nc = _build()
    res = run_bass_kernel_spmd(nc, in_maps, core_ids=list(range(NCORES)))
    LAST_RESULTS = res

    total = 0.0
    for r in res.results:
        total += float(r["loss_rows"].astype(np.float64).sum())
    return np.float32(total / N2)
